# revision 4
# baseline (speedup 1.0000x reference)
"""BERT attention layer (B=4, S=1024, H=1024, NH=16) on 8 TRN2 NeuronCores.

Sharding: core c -> batch b = c//2, sequence-half = c%2 (host rolls the
sequence for odd cores so one SPMD program serves all 8 cores).

Design (vs the 139us-sim / 184us-HW baseline):
- ACT is the pacing engine (64 exp tiles x ~1us = 66us).  Everything else
  is arranged to keep ACT gapless: chunked priority-ordered input DMAs
  (first exp ~7us), four score steps always in flight, the previous
  pair's normalize deferred until after the next pair's first scores.
- Both heads' ctx run aug'd DoubleRow matmuls at psum base 0 (DR dst
  must be base 0; the V ones-column lands each softmax denominator at
  psum row 64 for free).  Normalize = 2 bf16 psum->sbuf copies, 1 bf16
  reciprocal, 2 tiny bf16 broadcast matmuls, 2 multiplies; the odd
  half moves to ctxT rows 64:128 via an SBUF->SBUF DMA on the idle SP
  queue (engines cannot cross partitions; Pool cannot touch PSUM).
- Output projection split: j=0:512 contracts during pairs 4-7 into SBUF
  ya with residual+output-bias folded in and row-sums accumulated; only
  j=512:1024 remains after attention, in [P,2,QR] score-pool psums.
- bf16 LN tail split across engines: psum drains on DVE (st0/1, with
  accum_out row-sums) and ACT-copy+Pool-add (st2/3); sumsq via ACT
  Square+accum_out; (y-mu)*rstd, gamma, beta on DVE in bf16 (2x/4x
  modes); output tensor is bf16 (host converts to f32 in the gather).
"""

import sys

for _p in ("/opt/trn_rl_repo", "/root/.axon_site/_ro/trn_rl_repo"):
    if _p not in sys.path:
        sys.path.insert(0, _p)

import numpy as np

B, S, H, NH, HS = 4, 1024, 1024, 16, 64
P = 128
QR = 512          # query rows per core
EPS = 1e-12
N_CORES = 8
NP = NH // 2      # head pairs

# f32 aux rows
R_VB, R_OB = 0, 1
AUXF_ROWS = 2
# bf16 bundle rows: residual rows then gamma, beta
R_GAMMA = QR
R_BETA = QR + 1
XB_ROWS = QR + 2

_CACHE = {}


def _build_nc():
    import concourse.mybir as mybir
    import concourse.tile as tile
    from concourse import bacc

    f32 = mybir.dt.float32
    bf16 = mybir.dt.bfloat16
    f8 = mybir.dt.float8e4
    DR = mybir.MatmulPerfMode.DoubleRow
    Alu = mybir.AluOpType
    Act = mybir.ActivationFunctionType

    nc = bacc.Bacc("TRN2", target_bir_lowering=False)

    # ---- packed per-core DRAM I/O ----
    wx_d = nc.declare_dram_parameter("wx", [5 * H, S], f8, isOutput=False)
    # host-pretransposed [128, 3*8] qb/kb/mask block (contiguous lines)
    qkm_d = nc.declare_dram_parameter("qkm", [P, 3 * (H // P)], f32, isOutput=False)
    auxf_d = nc.declare_dram_parameter("auxf", [AUXF_ROWS, H], f32, isOutput=False)
    xb_d = nc.declare_dram_parameter("xb", [XB_ROWS, H], bf16, isOutput=False)
    out_d = nc.declare_dram_parameter("out", [QR, H], bf16, isOutput=True)

    xt_d = wx_d[0:H, :]
    qwt_d = wx_d[H : 2 * H, :]
    kwt_d = wx_d[2 * H : 3 * H, :]
    vwt_d = wx_d[3 * H : 4 * H, :]
    owt_d = wx_d[4 * H : 5 * H, :]

    KT8 = H // P   # 8 tiles along any 1024 dim
    ST = QR // P   # 4 row-tiles of the output

    with tile.TileContext(nc) as tc:
        with (
            tc.tile_pool(name="consts", bufs=1) as consts,
            tc.tile_pool(name="bigs", bufs=1) as bigs,
            tc.tile_pool(name="pers", bufs=2) as pers,
            tc.tile_pool(name="wbuf", bufs=4) as wbuf,
            tc.tile_pool(name="exps", bufs=2) as exps,
            tc.tile_pool(name="small", bufs=4) as small,
            tc.tile_pool(name="onorm", bufs=2) as onorm_pool,
            tc.tile_pool(name="norm", bufs=2) as norm_pool,
            tc.tile_pool(name="mm_ps", bufs=2, space="PSUM") as mm_ps,
            tc.tile_pool(name="sc_ps", bufs=2, space="PSUM") as sc_ps,
            tc.tile_pool(name="ctx_ps", bufs=1, space="PSUM") as ctx_ps,
            tc.tile_pool(name="dram", bufs=2, space="DRAM") as dram_pool,
        ):
            # ---- input DMAs.  HWDGE and the DMA engines are shared serial
            # resources; SP carries the critical sequence in strict
            # first-use order, ACT only the tiny qkm block.  128-col head
            # chunks of qwt/kwt unblock pair 0 (it only reads out-tile 0).
            qkm_sb = consts.tile([P, 3, KT8], f32)
            nc.scalar.dma_start(
                out=qkm_sb, in_=qkm_d.rearrange("p (r io) -> p r io", r=3)
            )
            qb_sb = qkm_sb[:, 0, :]
            kb_sb = qkm_sb[:, 1, :]
            mask_sb = qkm_sb[:, 2, :]

            # q/k projections and the sh=0 K half only read xt cols 0:512,
            # so that half loads first
            xt_sb = pers.tile([P, KT8, S], f8, tag="xt")
            xt_r = xt_d.rearrange("(io p) s -> p io s", p=P)
            qwt = wbuf.tile([P, KT8, H], f8, tag="wt", name="qwt")
            qwt_r = qwt_d.rearrange("(io p) o -> p io o", p=P)
            kwt = wbuf.tile([P, KT8, H], f8, tag="wt", name="kwt")
            kwt_r = kwt_d.rearrange("(io p) o -> p io o", p=P)
            nc.sync.dma_start(out=xt_sb[:, :, 0:QR], in_=xt_r[:, :, 0:QR])
            nc.sync.dma_start(out=qwt[:, :, 0:P], in_=qwt_r[:, :, 0:P])
            nc.sync.dma_start(out=kwt[:, :, 0:P], in_=kwt_r[:, :, 0:P])
            nc.sync.dma_start(out=kwt[:, :, P : 2 * P], in_=kwt_r[:, :, P : 2 * P])
            nc.sync.dma_start(out=qwt[:, :, P : 2 * P], in_=qwt_r[:, :, P : 2 * P])
            nc.sync.dma_start(out=xt_sb[:, :, QR:S], in_=xt_r[:, :, QR:S])
            nc.sync.dma_start(out=kwt[:, :, 2 * P : H], in_=kwt_r[:, :, 2 * P : H])
            nc.sync.dma_start(out=qwt[:, :, 2 * P : H], in_=qwt_r[:, :, 2 * P : H])
            vwt = wbuf.tile([P, KT8, H], f8, tag="wt", name="vwt")
            vwt_r = vwt_d.rearrange("(io p) o -> p io o", p=P)
            nc.sync.dma_start(out=vwt[:, :, 0:QR], in_=vwt_r[:, :, 0:QR])
            nc.sync.dma_start(out=vwt[:, :, QR:S], in_=vwt_r[:, :, QR:S])
            vbb_sb = consts.tile([P, H], f32)
            nc.sync.dma_start(
                out=vbb_sb, in_=auxf_d[R_VB : R_VB + 1, :].to_broadcast([P, H])
            )
            owt = wbuf.tile([P, KT8, H], f8, tag="wt", name="owt")
            owt_r = owt_d.rearrange("(io p) o -> p io o", p=P)
            for c in range(2):
                io = slice(4 * c, 4 * c + 4)
                nc.sync.dma_start(out=owt[:, io, :], in_=owt_r[:, io, :])
            # io-7 odd-head weight rows relocated to base partition 0 so the
            # tail can contract straight from the pair-7 staging tile
            owt7o = consts.tile([HS, H], f8)
            nc.sync.dma_start(out=owt7o, in_=owt_r[HS:P, 7, :])
            xres_r = xb_d[0:QR, :].rearrange("(st p) m -> p st m", p=P)
            xres_sb = bigs.tile([P, ST, H], bf16)
            for st in range(ST):
                nc.sync.dma_start(out=xres_sb[:, st, :], in_=xres_r[:, st, :])
            obb_sb = consts.tile([P, H], f32)
            nc.sync.dma_start(
                out=obb_sb, in_=auxf_d[R_OB : R_OB + 1, :].to_broadcast([P, H])
            )
            gbb_sb = consts.tile([P, H], bf16)
            nc.sync.dma_start(
                out=gbb_sb, in_=xb_d[R_GAMMA : R_GAMMA + 1, :].to_broadcast([P, H])
            )
            bbb_sb = consts.tile([P, H], bf16)
            nc.sync.dma_start(
                out=bbb_sb, in_=xb_d[R_BETA : R_BETA + 1, :].to_broadcast([P, H])
            )

            # ---- small constants ----
            eps_sb = consts.tile([P, 1], f32)
            nc.vector.memset(eps_sb, EPS)
            ones_bc = consts.tile([P, HS], bf16)
            nc.vector.memset(ones_bc, 1.0)
            rH = consts.tile([P, 1], f32)
            nc.vector.memset(rH, 1.0 / H)
            # warm the ACT exp table so the first real exp doesn't pay the
            # table load
            actwarm = consts.tile([P, 1], f32)
            nc.scalar.activation(out=actwarm, in_=eps_sb, func=Act.Exp)

            # ---- persistent big tensors ----
            QT = pers.tile([P, KT8, QR], bf16, tag="QT")   # Q^T  [o, q]
            KT = bigs.tile([P, KT8, S], bf16)              # K^T  [o, s]
            Vaug = bigs.tile([P, KT8, NH, HS + 1], f8)     # V [s, h, d(+ones)]
            ctxT = bigs.tile([P, KT8, QR], f8)             # ctx^T [j, q]
            ya = bigs.tile([P, ST, H], f32)                # first-half out-proj
            yasum = bigs.tile([P, ST, 2], f32)             # row-sums of ya halves
            xo = bigs.tile([P, ST, H], f32)                # residual + ob
            ybf = bigs.tile([P, ST, H], bf16)              # final y (bf16)

            nc.vector.memset(Vaug[:, :, :, HS : HS + 1], 1.0)

            def emit_v(st, oh):
                ps = mm_ps.tile([P, QR], f32, tag="mm", name="vps")
                for it in range(KT8 // 2):
                    nc.tensor.matmul(
                        ps,
                        xt_sb[:, 2 * it : 2 * it + 2, st * P : (st + 1) * P],
                        vwt[:, 2 * it : 2 * it + 2, oh * QR : (oh + 1) * QR],
                        start=(it == 0),
                        stop=(it == KT8 // 2 - 1),
                        perf_mode=DR,
                    )
                nc.vector.tensor_tensor(
                    out=Vaug[:, st, oh * (NH // 2) : (oh + 1) * (NH // 2), 0:HS],
                    in0=ps.rearrange("p (h d) -> p h d", d=HS),
                    in1=vbb_sb[:, oh * QR : (oh + 1) * QR].rearrange(
                        "p (h d) -> p h d", d=HS
                    ),
                    op=Alu.add,
                )

            def emit_q(ot):
                ps = mm_ps.tile([P, QR], f32, tag="mm", name="qps")
                for it in range(KT8 // 2):
                    nc.tensor.matmul(
                        ps,
                        qwt[:, 2 * it : 2 * it + 2, ot * P : (ot + 1) * P],
                        xt_sb[:, 2 * it : 2 * it + 2, 0:QR],
                        start=(it == 0),
                        stop=(it == KT8 // 2 - 1),
                        perf_mode=DR,
                    )
                nc.vector.tensor_scalar_add(
                    out=QT[:, ot, :], in0=ps, scalar1=qb_sb[:, ot : ot + 1]
                )

            def emit_kt(ot, sh):
                kps = mm_ps.tile([P, QR], f32, tag="mm", name="kps")
                for it in range(KT8 // 2):
                    nc.tensor.matmul(
                        kps,
                        kwt[:, 2 * it : 2 * it + 2, ot * P : (ot + 1) * P],
                        xt_sb[:, 2 * it : 2 * it + 2, sh * QR : (sh + 1) * QR],
                        start=(it == 0),
                        stop=(it == KT8 // 2 - 1),
                        perf_mode=DR,
                    )
                nc.vector.tensor_scalar_add(
                    out=KT[:, ot, sh * QR : (sh + 1) * QR],
                    in0=kps,
                    scalar1=kb_sb[:, ot : ot + 1],
                )

            def emit_xo(st):
                nc.gpsimd.tensor_tensor(
                    out=xo[:, st, :], in0=xres_sb[:, st, :], in1=obb_sb,
                    op=Alu.add,
                )

            def emit_oprojA(st, oh):
                ps = mm_ps.tile([P, QR], f32, tag="mm", name="oaps")
                for jo in range(2):
                    nc.tensor.matmul(
                        ps,
                        ctxT[:, 2 * jo : 2 * jo + 2, st * P : (st + 1) * P],
                        owt[:, 2 * jo : 2 * jo + 2, oh * QR : (oh + 1) * QR],
                        start=(jo == 0),
                        stop=(jo == 1),
                        perf_mode=DR,
                    )
                nc.vector.scalar_tensor_tensor(
                    out=ya[:, st, oh * QR : (oh + 1) * QR],
                    in0=ps,
                    scalar=0.0,
                    in1=xo[:, st, oh * QR : (oh + 1) * QR],
                    op0=Alu.add,
                    op1=Alu.add,
                    accum_out=yasum[:, st, oh : oh + 1],
                )

            def alloc_pair():
                expS = exps.tile([P, KT8, 2, QR], f8, tag="expS", name="expS")
                cpsA = ctx_ps.tile([P, QR], f32, tag="ctxA", name="cpsA")
                cpsB = ctx_ps.tile([P, QR], f32, tag="ctxB", name="cpsB")
                return expS, cpsA, cpsB

            def emit_scores(oo, so, expS):
                sps = sc_ps.tile([P, 2, QR], f32, tag="sc", name="sps")
                for j in range(2):
                    po = j * HS
                    nc.tensor.matmul(
                        sps[:, j, :],
                        KT[po : po + HS, oo, so * P : (so + 1) * P],
                        QT[po : po + HS, oo, :],
                        start=True,
                        stop=True,
                    )
                nc.scalar.activation(
                    out=expS[:, so, :, :],
                    in_=sps,
                    func=Act.Exp,
                    bias=mask_sb[:, so : so + 1],
                    scale=0.125,
                )

            def emit_ctx_pair(oo, t, expS, cpsA, cpsB):
                first, last = t == 0, t == KT8 // 2 - 1
                # both heads: aug'd DoubleRow at base 0 (DR dst must be base
                # 0); the ones column lands each head's softmax denominator
                # at psum row 64 for free
                for j in range(2):
                    nc.tensor.matmul(
                        (cpsA, cpsB)[j][0 : HS + 1, :],
                        Vaug[:, 2 * t : 2 * t + 2, 2 * oo + j, :],
                        expS[:, 2 * t : 2 * t + 2, j, :],
                        start=first, stop=last, perf_mode=DR,
                    )

            def emit_norm(oo, cpsA, cpsB):
                # copy both heads' ctx+denominator rows to SBUF (bf16),
                # reciprocal, broadcast via tiny bf16 matmuls into mm-pool
                # psums, multiply; the odd half moves to ctxT rows 64:128
                # via an SBUF->SBUF DMA on the otherwise-idle SP queue
                cc = norm_pool.tile([HS + 1, 2, QR], bf16, tag="cc")
                nc.vector.tensor_copy(out=cc[:, 0, :], in_=cpsA[0 : HS + 1, :])
                nc.vector.tensor_copy(out=cc[:, 1, :], in_=cpsB[0 : HS + 1, :])
                rdt = norm_pool.tile([P, 2, QR], bf16, tag="rd2")
                with nc.allow_low_precision(
                    "softmax denominators; bf16 recip adds <0.5% rel err"
                ):
                    nc.vector.reciprocal(
                        out=rdt[HS : HS + 1, :, :], in_=cc[HS : HS + 1, :, :]
                    )
                bcA = mm_ps.tile([P, QR], f32, tag="mm", name="bcA")
                bcB = mm_ps.tile([P, QR], f32, tag="mm", name="bcB")
                nc.tensor.matmul(
                    bcA[0:HS, :], ones_bc[HS : HS + 1, :], rdt[HS : HS + 1, 0, :],
                    start=True, stop=True,
                )
                nc.tensor.matmul(
                    bcB[0:HS, :], ones_bc[HS : HS + 1, :], rdt[HS : HS + 1, 1, :],
                    start=True, stop=True,
                )
                nc.vector.tensor_tensor(
                    out=ctxT[0:HS, oo, :], in0=cc[0:HS, 0, :], in1=bcA[0:HS, :],
                    op=Alu.mult,
                )
                stage = norm_pool.tile([HS, QR], f8, tag="stage", bufs=3)
                nc.vector.tensor_tensor(
                    out=stage, in0=cc[0:HS, 1, :], in1=bcB[0:HS, :],
                    op=Alu.mult,
                )
                if oo < NP - 1:
                    nc.sync.dma_start(out=ctxT[HS:P, oo, :], in_=stage)
                return stage

            # ---- attention ----
            emit_q(0)
            emit_kt(0, 0)

            # PE queue is in-order: q(0)/kt(0,0), then the first four score
            # steps (they only need out-tile 0), then the rest of the
            # preamble that waits on later DMA chunks
            cur = alloc_pair()
            for so in range(4):
                emit_scores(0, so, cur[0])
            emit_kt(0, 1)
            emit_q(1)
            pending_norm = None

            extra = {}

            def add_extra(oo, fn, args, front=False):
                lst = extra.setdefault(oo, [])
                lst.insert(0, (fn, args)) if front else lst.append((fn, args))

            for i, st in enumerate(range(KT8)):
                add_extra(1 + i // 3, emit_v, (st, 1))
            # K/Q for the NEXT pairs must be emitted BEFORE that pair's
            # pre-emitted score matmuls (program order defines data flow),
            # so they go at the FRONT of the work list (chunk 0)
            for oo in range(1, NP - 2):
                add_extra(oo, emit_q, (oo + 2,), front=True)
            for oo in range(1, NP - 1):
                add_extra(oo, emit_kt, (oo + 1, 1), front=True)
                add_extra(oo, emit_kt, (oo + 1, 0), front=True)
            for st in range(ST):
                for oh in range(2):
                    add_extra(4 + st, emit_oprojA, (st, oh))
            # pair 0 interleave is hand-ordered: ctx(0,t) needs emit_v(2t,0)
            # and emit_v(2t+1,0) by chunk t, and kt(1,0) must precede the
            # pre-emitted pair-1 scores
            extra[0] = [
                (emit_v, (0, 0)), (emit_v, (1, 0)), (emit_kt, (1, 0)),
                (emit_v, (2, 0)), (emit_v, (3, 0)), (emit_kt, (1, 1)),
                (emit_v, (4, 0)), (emit_v, (5, 0)), (emit_q, (2,)),
                (emit_v, (6, 0)), (emit_v, (7, 0)),
            ]

            for oo in range(NP):
                expS, cpsA, cpsB = cur
                work = list(extra.get(oo, []))
                if oo in (2, 3):
                    work.append((emit_xo, (2 * (oo - 2),)))
                    work.append((emit_xo, (2 * (oo - 2) + 1,)))
                chunks = [[] for _ in range(4)]
                for i, w in enumerate(work):
                    chunks[i * 4 // max(len(work), 1)].append(w)

                for t in range(KT8 // 2):
                    if t < 2:
                        emit_scores(oo, 2 * t + 4, expS)
                        emit_scores(oo, 2 * t + 5, expS)
                    elif oo + 1 < NP:
                        if t == 2:
                            nxt = alloc_pair()
                        emit_scores(oo + 1, 2 * (t - 2), nxt[0])
                        emit_scores(oo + 1, 2 * (t - 2) + 1, nxt[0])
                    for fn, args in chunks[t]:
                        fn(*args)
                    if t == 0 and pending_norm is not None:
                        # previous pair's normalize after this pair's first
                        # scores AND chunk-0 work (kt/q emits), but before
                        # ctx(0) which reuses the psum banks
                        emit_norm(*pending_norm)
                    emit_ctx_pair(oo, t, expS, cpsA, cpsB)

                pending_norm = (oo, cpsA, cpsB)
                if oo + 1 < NP:
                    cur = nxt
            stage7 = emit_norm(*pending_norm)

            # warm the Square/Sqrt tables right after the last exp (the
            # data dep on expS pins these behind the attention in ACT order)
            sqwarm = consts.tile([P, 1], f32)
            nc.scalar.activation(
                out=sqwarm, in_=expS[:, KT8 - 1, 1, 0:1], func=Act.Square
            )
            nc.scalar.activation(out=sqwarm, in_=sqwarm, func=Act.Sqrt)

            # ---- second-half output projection + layernorm tail ----
            # halfB matmuls land in sc_ps [P,2,QR] tiles; DR(4,5) groups are
            # emitted before DR(6,7) so only the latter waits on the pair-7
            # stage move.  Drains: DVE w/ accum (st0/1), ACT-copy+Pool-add
            # (st2/3, Pool cannot touch PSUM; musum from precomputed ya
            # sums).  sumsq via ACT Square+accum.  Output bf16.
            for st in range(ST):
                sps = sc_ps.tile([P, 2, QR], f32, tag="sc", name="obps")
                for oh in range(2):
                    nc.tensor.matmul(
                        sps[:, oh, :],
                        ctxT[:, 4:6, st * P : (st + 1) * P],
                        owt[:, 4:6, oh * QR : (oh + 1) * QR],
                        start=True, stop=False, perf_mode=DR,
                    )
                for oh in range(2):
                    nc.tensor.matmul(
                        sps[:, oh, :],
                        ctxT[:, 6, st * P : (st + 1) * P],
                        owt[:, 6, oh * QR : (oh + 1) * QR],
                        start=False, stop=False,
                    )
                for oh in range(2):
                    nc.tensor.matmul(
                        sps[:, oh, :],
                        ctxT[0:HS, 7, st * P : (st + 1) * P],
                        owt[0:HS, 7, oh * QR : (oh + 1) * QR],
                        start=False, stop=False,
                    )
                for oh in range(2):
                    nc.tensor.matmul(
                        sps[:, oh, :],
                        stage7[:, st * P : (st + 1) * P],
                        owt7o[:, oh * QR : (oh + 1) * QR],
                        start=False, stop=True,
                    )
                musum = small.tile([P, 1], f32, tag="musum")
                if st < 2:
                    nc.vector.scalar_tensor_tensor(
                        out=ybf[:, st, :].rearrange("p (j q) -> p j q", q=QR),
                        in0=sps,
                        scalar=0.0,
                        in1=ya[:, st, :].rearrange("p (j q) -> p j q", q=QR),
                        op0=Alu.add,
                        op1=Alu.add,
                        accum_out=musum,
                    )
                else:
                    ysb = onorm_pool.tile([P, H], f32, tag="ysb")
                    pbsum = small.tile([P, 1], f32, tag="pbsum")
                    nc.scalar.activation(
                        out=ysb.rearrange("p (j q) -> p j q", q=QR), in_=sps,
                        func=Act.Copy, accum_out=pbsum,
                    )
                    nc.gpsimd.tensor_tensor(
                        out=ybf[:, st, :], in0=ysb, in1=ya[:, st, :],
                        op=Alu.add,
                    )
                    nc.vector.tensor_scalar(
                        out=musum, in0=yasum[:, st, 0:1],
                        scalar1=yasum[:, st, 1:2],
                        scalar2=pbsum, op0=Alu.add, op1=Alu.add,
                    )

                scr = onorm_pool.tile([P, H], bf16, tag="scr")
                ysq = small.tile([P, 1], f32, tag="ysq")
                nc.scalar.activation(
                    out=scr, in_=ybf[:, st, :], func=Act.Square,
                    accum_out=ysq,
                )
                mu = small.tile([P, 1], f32, tag="mu")
                nc.vector.tensor_scalar(
                    out=mu, in0=musum, scalar1=rH, scalar2=None, op0=Alu.mult,
                )
                mu2 = small.tile([P, 1], f32, tag="mu2")
                nc.vector.tensor_tensor(out=mu2, in0=mu, in1=mu, op=Alu.mult)
                var = small.tile([P, 1], f32, tag="var")
                nc.vector.tensor_scalar(
                    out=var, in0=ysq, scalar1=rH, scalar2=mu2,
                    op0=Alu.mult, op1=Alu.subtract,
                )
                rstd = small.tile([P, 1], f32, tag="rstd")
                nc.scalar.activation(
                    out=rstd, in_=var, func=Act.Sqrt, bias=eps_sb, scale=1.0
                )
                nc.vector.reciprocal(out=rstd, in_=rstd)
                on = onorm_pool.tile([P, H], bf16, tag="on")
                nc.vector.tensor_scalar(
                    out=on, in0=ybf[:, st, :], scalar1=mu, scalar2=rstd,
                    op0=Alu.subtract, op1=Alu.mult,
                )
                nc.vector.tensor_tensor(out=on, in0=on, in1=gbb_sb, op=Alu.mult)
                onf = onorm_pool.tile([P, H], bf16, tag="onf")
                nc.vector.tensor_tensor(out=onf, in0=on, in1=bbb_sb, op=Alu.add)
                oeng = nc.scalar if st % 2 else nc.sync
                oeng.dma_start(
                    out=out_d.rearrange("(st p) m -> p st m", p=P)[:, st, :],
                    in_=onf,
                )

    nc.compile()
    return nc


def _get_nc():
    if "nc" not in _CACHE:
        _CACHE["nc"] = _build_nc()
    return _CACHE["nc"]


def _make_in_maps(inputs):
    import ml_dtypes

    f8 = ml_dtypes.float8_e4m3
    bf = ml_dtypes.bfloat16
    hs = np.asarray(inputs["hidden_states"], dtype=np.float32).reshape(B, S, H)
    am = np.asarray(inputs["attention_mask"], dtype=np.float32).reshape(B, S)

    # shared fp8 weight block [4096, 1024]: qw^T, kw^T, vw^T, ow^T
    wblk = np.empty((4 * H, H), dtype=f8)
    for i, nm in enumerate(("qw", "kw", "vw", "ow")):
        wblk[i * H : (i + 1) * H] = np.asarray(inputs[nm], np.float32).T.astype(f8)

    auxf_shared = np.zeros((AUXF_ROWS, H), dtype=np.float32)
    auxf_shared[R_VB] = np.asarray(inputs["vb"], np.float32)
    auxf_shared[R_OB] = np.asarray(inputs["ob"], np.float32)
    qb = np.asarray(inputs["qb"], np.float32)
    kb = np.asarray(inputs["kb"], np.float32)
    gb = np.asarray(inputs["gamma"], np.float32).astype(bf)
    bb = np.asarray(inputs["beta"], np.float32).astype(bf)

    in_maps = []
    for c in range(N_CORES):
        b, half = divmod(c, 2)
        x = hs[b]
        m = am[b]
        if half:
            x = np.roll(x, -QR, axis=0)
            m = np.roll(m, -QR)
        wx = np.empty((5 * H, S), dtype=f8)
        wx[0:H] = x.T.astype(f8)
        wx[H:] = wblk
        # pretransposed [128, 3*8] qkm block: [p, r, io] = row_r[io*128+p]
        qkm = np.empty((P, 3, H // P), dtype=np.float32)
        qkm[:, 0, :] = qb.reshape(H // P, P).T
        qkm[:, 1, :] = kb.reshape(H // P, P).T
        qkm[:, 2, :] = m.reshape(H // P, P).T
        xb = np.empty((XB_ROWS, H), dtype=bf)
        xb[0:QR] = x[:QR].astype(bf)
        xb[R_GAMMA] = gb
        xb[R_BETA] = bb
        in_maps.append({
            "wx": wx,
            "qkm": qkm.reshape(P, 3 * (H // P)),
            "auxf": auxf_shared,
            "xb": xb,
        })
    return in_maps


def _gather(results):
    out = np.empty((B, S, H), dtype=np.float32)
    for c in range(N_CORES):
        b, half = divmod(c, 2)
        out[b, half * QR : (half + 1) * QR, :] = results[c]["out"]
    return out


def run_on_hw(inputs, **kwargs):
    """Run on the 8 NeuronCores; returns (full_output, BassKernelResults)."""
    from concourse import bass_utils

    nc = _get_nc()
    in_maps = _make_in_maps(inputs)
    res = bass_utils.run_bass_kernel_spmd(
        nc, in_maps, core_ids=list(range(N_CORES)), **kwargs
    )
    return _gather(res.results), res


def kernel(**inputs) -> np.ndarray:
    out, _ = run_on_hw(inputs)
    return out


# revision 5
# speedup vs baseline: 1.0586x; 1.0586x over previous
"""BERT attention layer (B=4, S=1024, H=1024, NH=16) on 8 TRN2 NeuronCores.

Sharding: core c -> batch b = c//2, sequence-half = c%2 (host rolls the
sequence for odd cores so one SPMD program serves all 8 cores).

Design (vs the 139us-sim / 184us-HW baseline):
- ACT is the pacing engine (64 exp tiles x ~1us = 66us).  Everything else
  is arranged to keep ACT gapless: chunked priority-ordered input DMAs
  (first exp ~7us), four score steps always in flight, the previous
  pair's normalize deferred until after the next pair's first scores.
- Both heads' ctx run aug'd DoubleRow matmuls at psum base 0 (DR dst
  must be base 0; the V ones-column lands each softmax denominator at
  psum row 64 for free).  Normalize = 2 bf16 psum->sbuf copies, 1 bf16
  reciprocal, 2 tiny bf16 broadcast matmuls, 2 multiplies; the odd
  half moves to ctxT rows 64:128 via an SBUF->SBUF DMA on the idle SP
  queue (engines cannot cross partitions; Pool cannot touch PSUM).
- Output projection split: j=0:512 contracts during pairs 4-7 into SBUF
  ya with residual+output-bias folded in and row-sums accumulated; only
  j=512:1024 remains after attention, in [P,2,QR] score-pool psums.
- bf16 LN tail split across engines: psum drains on DVE (st0/1, with
  accum_out row-sums) and ACT-copy+Pool-add (st2/3); sumsq via ACT
  Square+accum_out; (y-mu)*rstd, gamma, beta on DVE in bf16 (2x/4x
  modes); output tensor is bf16 (host converts to f32 in the gather).
"""

import sys

for _p in ("/opt/trn_rl_repo", "/root/.axon_site/_ro/trn_rl_repo"):
    if _p not in sys.path:
        sys.path.insert(0, _p)

import numpy as np

B, S, H, NH, HS = 4, 1024, 1024, 16, 64
P = 128
QR = 512          # query rows per core
EPS = 1e-12
N_CORES = 8
NP = NH // 2      # head pairs

# f32 aux rows
R_VB, R_OB = 0, 1
AUXF_ROWS = 2
# bf16 bundle rows: residual rows then gamma, beta
R_GAMMA = QR
R_BETA = QR + 1
XB_ROWS = QR + 2

_CACHE = {}


def _build_nc():
    import concourse.mybir as mybir
    import concourse.tile as tile
    from concourse import bacc

    f32 = mybir.dt.float32
    bf16 = mybir.dt.bfloat16
    f8 = mybir.dt.float8e4
    DR = mybir.MatmulPerfMode.DoubleRow
    Alu = mybir.AluOpType
    Act = mybir.ActivationFunctionType

    nc = bacc.Bacc("TRN2", target_bir_lowering=False)

    # ---- packed per-core DRAM I/O ----
    wx_d = nc.declare_dram_parameter("wx", [5 * H, S], f8, isOutput=False)
    # host-pretransposed [128, 3*8] qb/kb/mask block (contiguous lines)
    qkm_d = nc.declare_dram_parameter("qkm", [P, 3 * (H // P)], f32, isOutput=False)
    auxf_d = nc.declare_dram_parameter("auxf", [AUXF_ROWS, H], f32, isOutput=False)
    xb_d = nc.declare_dram_parameter("xb", [XB_ROWS, H], bf16, isOutput=False)
    out_d = nc.declare_dram_parameter("out", [QR, H], bf16, isOutput=True)

    xt_d = wx_d[0:H, :]
    qwt_d = wx_d[H : 2 * H, :]
    kwt_d = wx_d[2 * H : 3 * H, :]
    vwt_d = wx_d[3 * H : 4 * H, :]
    owt_d = wx_d[4 * H : 5 * H, :]

    KT8 = H // P   # 8 tiles along any 1024 dim
    ST = QR // P   # 4 row-tiles of the output

    with tile.TileContext(nc) as tc:
        with (
            tc.tile_pool(name="consts", bufs=1) as consts,
            tc.tile_pool(name="bigs", bufs=1) as bigs,
            tc.tile_pool(name="pers", bufs=2) as pers,
            tc.tile_pool(name="wbuf", bufs=4) as wbuf,
            tc.tile_pool(name="exps", bufs=2) as exps,
            tc.tile_pool(name="small", bufs=4) as small,
            tc.tile_pool(name="onorm", bufs=2) as onorm_pool,
            tc.tile_pool(name="norm", bufs=2) as norm_pool,
            tc.tile_pool(name="mm_ps", bufs=2, space="PSUM") as mm_ps,
            tc.tile_pool(name="sc_ps", bufs=2, space="PSUM") as sc_ps,
            tc.tile_pool(name="ctx_ps", bufs=1, space="PSUM") as ctx_ps,
            tc.tile_pool(name="dram", bufs=2, space="DRAM") as dram_pool,
        ):
            # ---- input DMAs.  HWDGE and the DMA engines are shared serial
            # resources; SP carries the critical sequence in strict
            # first-use order, ACT only the tiny qkm block.  128-col head
            # chunks of qwt/kwt unblock pair 0 (it only reads out-tile 0).
            qkm_sb = consts.tile([P, 3, KT8], f32)
            nc.scalar.dma_start(
                out=qkm_sb, in_=qkm_d.rearrange("p (r io) -> p r io", r=3)
            )
            qb_sb = qkm_sb[:, 0, :]
            kb_sb = qkm_sb[:, 1, :]
            mask_sb = qkm_sb[:, 2, :]

            # q/k projections and the sh=0 K half only read xt cols 0:512,
            # so that half loads first
            xt_sb = pers.tile([P, KT8, S], f8, tag="xt")
            xt_r = xt_d.rearrange("(io p) s -> p io s", p=P)
            qwt = wbuf.tile([P, KT8, H], f8, tag="wt", name="qwt")
            qwt_r = qwt_d.rearrange("(io p) o -> p io o", p=P)
            kwt = wbuf.tile([P, KT8, H], f8, tag="wt", name="kwt")
            kwt_r = kwt_d.rearrange("(io p) o -> p io o", p=P)
            nc.sync.dma_start(out=xt_sb[:, :, 0:QR], in_=xt_r[:, :, 0:QR])
            nc.sync.dma_start(out=qwt[:, :, 0:P], in_=qwt_r[:, :, 0:P])
            nc.sync.dma_start(out=kwt[:, :, 0:P], in_=kwt_r[:, :, 0:P])
            nc.sync.dma_start(out=kwt[:, :, P : 2 * P], in_=kwt_r[:, :, P : 2 * P])
            nc.sync.dma_start(out=qwt[:, :, P : 2 * P], in_=qwt_r[:, :, P : 2 * P])
            nc.sync.dma_start(out=xt_sb[:, :, QR:S], in_=xt_r[:, :, QR:S])
            nc.sync.dma_start(out=kwt[:, :, 2 * P : H], in_=kwt_r[:, :, 2 * P : H])
            nc.sync.dma_start(out=qwt[:, :, 2 * P : H], in_=qwt_r[:, :, 2 * P : H])
            vwt = wbuf.tile([P, KT8, H], f8, tag="wt", name="vwt")
            vwt_r = vwt_d.rearrange("(io p) o -> p io o", p=P)
            nc.sync.dma_start(out=vwt[:, :, 0:QR], in_=vwt_r[:, :, 0:QR])
            nc.sync.dma_start(out=vwt[:, :, QR:S], in_=vwt_r[:, :, QR:S])
            vbb_sb = consts.tile([P, H], f32)
            nc.sync.dma_start(
                out=vbb_sb, in_=auxf_d[R_VB : R_VB + 1, :].to_broadcast([P, H])
            )
            owt = wbuf.tile([P, KT8, H], f8, tag="wt", name="owt")
            owt_r = owt_d.rearrange("(io p) o -> p io o", p=P)
            for c in range(2):
                io = slice(4 * c, 4 * c + 4)
                nc.sync.dma_start(out=owt[:, io, :], in_=owt_r[:, io, :])
            # io-7 odd-head weight rows relocated to base partition 0 so the
            # tail can contract straight from the pair-7 staging tile
            owt7o = consts.tile([HS, H], f8)
            nc.sync.dma_start(out=owt7o, in_=owt_r[HS:P, 7, :])
            xres_r = xb_d[0:QR, :].rearrange("(st p) m -> p st m", p=P)
            xres_sb = bigs.tile([P, ST, H], bf16)
            for st in range(ST):
                nc.sync.dma_start(out=xres_sb[:, st, :], in_=xres_r[:, st, :])
            obb_sb = consts.tile([P, H], f32)
            nc.sync.dma_start(
                out=obb_sb, in_=auxf_d[R_OB : R_OB + 1, :].to_broadcast([P, H])
            )
            gbb_sb = consts.tile([P, H], bf16)
            nc.sync.dma_start(
                out=gbb_sb, in_=xb_d[R_GAMMA : R_GAMMA + 1, :].to_broadcast([P, H])
            )
            bbb_sb = consts.tile([P, H], bf16)
            nc.sync.dma_start(
                out=bbb_sb, in_=xb_d[R_BETA : R_BETA + 1, :].to_broadcast([P, H])
            )

            # ---- small constants ----
            eps_sb = consts.tile([P, 1], f32)
            nc.vector.memset(eps_sb, EPS)
            ones_bc = consts.tile([P, HS], bf16)
            nc.vector.memset(ones_bc, 1.0)
            rH = consts.tile([P, 1], f32)
            nc.vector.memset(rH, 1.0 / H)
            # warm the ACT exp table so the first real exp doesn't pay the
            # table load
            actwarm = consts.tile([P, 1], f32)
            nc.scalar.activation(out=actwarm, in_=eps_sb, func=Act.Exp)

            # ---- persistent big tensors ----
            QT = pers.tile([P, KT8, QR], bf16, tag="QT")   # Q^T  [o, q]
            KT = bigs.tile([P, KT8, S], bf16)              # K^T  [o, s]
            Vaug = bigs.tile([P, KT8, NH, HS + 1], f8)     # V [s, h, d(+ones)]
            ctxT = bigs.tile([P, KT8, QR], f8)             # ctx^T [j, q]
            ya = bigs.tile([P, ST, H], f32)                # first-half out-proj
            yasum = bigs.tile([P, ST, 2], f32)             # row-sums of ya halves
            xo = bigs.tile([P, ST, H], f32)                # residual + ob
            ybf = bigs.tile([P, ST, H], bf16)              # final y (bf16)

            nc.vector.memset(Vaug[:, :, :, HS : HS + 1], 1.0)

            def emit_v(st, oh):
                ps = mm_ps.tile([P, QR], f32, tag="mm", name="vps")
                for it in range(KT8 // 2):
                    nc.tensor.matmul(
                        ps,
                        xt_sb[:, 2 * it : 2 * it + 2, st * P : (st + 1) * P],
                        vwt[:, 2 * it : 2 * it + 2, oh * QR : (oh + 1) * QR],
                        start=(it == 0),
                        stop=(it == KT8 // 2 - 1),
                        perf_mode=DR,
                    )
                nc.vector.tensor_tensor(
                    out=Vaug[:, st, oh * (NH // 2) : (oh + 1) * (NH // 2), 0:HS],
                    in0=ps.rearrange("p (h d) -> p h d", d=HS),
                    in1=vbb_sb[:, oh * QR : (oh + 1) * QR].rearrange(
                        "p (h d) -> p h d", d=HS
                    ),
                    op=Alu.add,
                )

            def emit_q(ot):
                ps = mm_ps.tile([P, QR], f32, tag="mm", name="qps")
                for it in range(KT8 // 2):
                    nc.tensor.matmul(
                        ps,
                        qwt[:, 2 * it : 2 * it + 2, ot * P : (ot + 1) * P],
                        xt_sb[:, 2 * it : 2 * it + 2, 0:QR],
                        start=(it == 0),
                        stop=(it == KT8 // 2 - 1),
                        perf_mode=DR,
                    )
                nc.vector.tensor_scalar_add(
                    out=QT[:, ot, :], in0=ps, scalar1=qb_sb[:, ot : ot + 1]
                )

            def emit_kt(ot, sh):
                kps = mm_ps.tile([P, QR], f32, tag="mm", name="kps")
                for it in range(KT8 // 2):
                    nc.tensor.matmul(
                        kps,
                        kwt[:, 2 * it : 2 * it + 2, ot * P : (ot + 1) * P],
                        xt_sb[:, 2 * it : 2 * it + 2, sh * QR : (sh + 1) * QR],
                        start=(it == 0),
                        stop=(it == KT8 // 2 - 1),
                        perf_mode=DR,
                    )
                nc.vector.tensor_scalar_add(
                    out=KT[:, ot, sh * QR : (sh + 1) * QR],
                    in0=kps,
                    scalar1=kb_sb[:, ot : ot + 1],
                )

            def emit_xo(st):
                nc.gpsimd.tensor_tensor(
                    out=xo[:, st, :], in0=xres_sb[:, st, :], in1=obb_sb,
                    op=Alu.add,
                )

            def emit_oprojA(st, oh):
                ps = mm_ps.tile([P, QR], f32, tag="mm", name="oaps")
                for jo in range(2):
                    nc.tensor.matmul(
                        ps,
                        ctxT[:, 2 * jo : 2 * jo + 2, st * P : (st + 1) * P],
                        owt[:, 2 * jo : 2 * jo + 2, oh * QR : (oh + 1) * QR],
                        start=(jo == 0),
                        stop=(jo == 1),
                        perf_mode=DR,
                    )
                nc.vector.scalar_tensor_tensor(
                    out=ya[:, st, oh * QR : (oh + 1) * QR],
                    in0=ps,
                    scalar=0.0,
                    in1=xo[:, st, oh * QR : (oh + 1) * QR],
                    op0=Alu.add,
                    op1=Alu.add,
                    accum_out=yasum[:, st, oh : oh + 1],
                )

            def alloc_pair():
                expS = exps.tile([P, KT8, 2, QR], f8, tag="expS", name="expS")
                cpsA = ctx_ps.tile([P, QR], f32, tag="ctxA", name="cpsA")
                cpsB = ctx_ps.tile([P, QR], f32, tag="ctxB", name="cpsB")
                return expS, cpsA, cpsB

            def emit_scores(oo, so, expS):
                sps = sc_ps.tile([P, 2, QR], f32, tag="sc", name="sps")
                for j in range(2):
                    po = j * HS
                    nc.tensor.matmul(
                        sps[:, j, :],
                        KT[po : po + HS, oo, so * P : (so + 1) * P],
                        QT[po : po + HS, oo, :],
                        start=True,
                        stop=True,
                    )
                nc.scalar.activation(
                    out=expS[:, so, :, :],
                    in_=sps,
                    func=Act.Exp,
                    bias=mask_sb[:, so : so + 1],
                    scale=0.125,
                )

            def emit_ctx_pair(oo, t, expS, cpsA, cpsB):
                first, last = t == 0, t == KT8 // 2 - 1
                # both heads: aug'd DoubleRow at base 0 (DR dst must be base
                # 0); the ones column lands each head's softmax denominator
                # at psum row 64 for free
                for j in range(2):
                    nc.tensor.matmul(
                        (cpsA, cpsB)[j][0 : HS + 1, :],
                        Vaug[:, 2 * t : 2 * t + 2, 2 * oo + j, :],
                        expS[:, 2 * t : 2 * t + 2, j, :],
                        start=first, stop=last, perf_mode=DR,
                    )

            def emit_norm(oo, cpsA, cpsB):
                # copy both heads' ctx+denominator rows to SBUF (bf16),
                # reciprocal, broadcast via tiny bf16 matmuls into mm-pool
                # psums, multiply; the odd half moves to ctxT rows 64:128
                # via an SBUF->SBUF DMA on the otherwise-idle SP queue
                cc = norm_pool.tile([HS + 1, 2, QR], bf16, tag="cc")
                nc.vector.tensor_copy(out=cc[:, 0, :], in_=cpsA[0 : HS + 1, :])
                nc.vector.tensor_copy(out=cc[:, 1, :], in_=cpsB[0 : HS + 1, :])
                rdt = norm_pool.tile([P, 2, QR], bf16, tag="rd2")
                with nc.allow_low_precision(
                    "softmax denominators; bf16 recip adds <0.5% rel err"
                ):
                    nc.vector.reciprocal(
                        out=rdt[HS : HS + 1, :, :], in_=cc[HS : HS + 1, :, :]
                    )
                bcA = mm_ps.tile([P, QR], f32, tag="mm", name="bcA")
                bcB = mm_ps.tile([P, QR], f32, tag="mm", name="bcB")
                nc.tensor.matmul(
                    bcA[0:HS, :], ones_bc[HS : HS + 1, :], rdt[HS : HS + 1, 0, :],
                    start=True, stop=True,
                )
                nc.tensor.matmul(
                    bcB[0:HS, :], ones_bc[HS : HS + 1, :], rdt[HS : HS + 1, 1, :],
                    start=True, stop=True,
                )
                nc.vector.tensor_tensor(
                    out=ctxT[0:HS, oo, :], in0=cc[0:HS, 0, :], in1=bcA[0:HS, :],
                    op=Alu.mult,
                )
                stage = norm_pool.tile([HS, QR], f8, tag="stage", bufs=3)
                nc.vector.tensor_tensor(
                    out=stage, in0=cc[0:HS, 1, :], in1=bcB[0:HS, :],
                    op=Alu.mult,
                )
                if oo < NP - 1:
                    nc.sync.dma_start(out=ctxT[HS:P, oo, :], in_=stage)
                return stage

            # ---- attention ----
            emit_q(0)
            emit_kt(0, 0)

            # PE queue is in-order: q(0)/kt(0,0), then the first four score
            # steps (they only need out-tile 0), then the rest of the
            # preamble that waits on later DMA chunks
            cur = alloc_pair()
            for so in range(4):
                emit_scores(0, so, cur[0])
            emit_kt(0, 1)
            emit_q(1)
            pending_norm = None

            extra = {}

            def add_extra(oo, fn, args, front=False):
                lst = extra.setdefault(oo, [])
                lst.insert(0, (fn, args)) if front else lst.append((fn, args))

            for i, st in enumerate(range(KT8)):
                add_extra(1 + i // 3, emit_v, (st, 1))
            # K/Q for the NEXT pairs must be emitted BEFORE that pair's
            # pre-emitted score matmuls (program order defines data flow),
            # so they go at the FRONT of the work list (chunk 0)
            for oo in range(1, NP - 2):
                add_extra(oo, emit_q, (oo + 2,), front=True)
            for oo in range(1, NP - 1):
                add_extra(oo, emit_kt, (oo + 1, 1), front=True)
                add_extra(oo, emit_kt, (oo + 1, 0), front=True)
            for st in range(ST):
                for oh in range(2):
                    add_extra(4 + st, emit_oprojA, (st, oh))
            # pair 0 interleave is hand-ordered: ctx(0,t) needs emit_v(2t,0)
            # and emit_v(2t+1,0) by chunk t, and kt(1,0) must precede the
            # pre-emitted pair-1 scores
            extra[0] = [
                (emit_v, (0, 0)), (emit_v, (1, 0)), (emit_kt, (1, 0)),
                (emit_v, (2, 0)), (emit_v, (3, 0)), (emit_kt, (1, 1)),
                (emit_v, (4, 0)), (emit_v, (5, 0)), (emit_q, (2,)),
                (emit_v, (6, 0)), (emit_v, (7, 0)),
            ]

            for oo in range(NP):
                expS, cpsA, cpsB = cur
                work = list(extra.get(oo, []))
                if oo in (2, 3):
                    work.append((emit_xo, (2 * (oo - 2),)))
                    work.append((emit_xo, (2 * (oo - 2) + 1,)))
                chunks = [[] for _ in range(4)]
                for i, w in enumerate(work):
                    chunks[i * 4 // max(len(work), 1)].append(w)

                for t in range(KT8 // 2):
                    if t < 2:
                        emit_scores(oo, 2 * t + 4, expS)
                        emit_scores(oo, 2 * t + 5, expS)
                    elif oo + 1 < NP:
                        if t == 2:
                            nxt = alloc_pair()
                        emit_scores(oo + 1, 2 * (t - 2), nxt[0])
                        emit_scores(oo + 1, 2 * (t - 2) + 1, nxt[0])
                    for fn, args in chunks[t]:
                        fn(*args)
                    if t == 0 and pending_norm is not None:
                        # previous pair's normalize after this pair's first
                        # scores AND chunk-0 work (kt/q emits), but before
                        # ctx(0) which reuses the psum banks
                        emit_norm(*pending_norm)
                    emit_ctx_pair(oo, t, expS, cpsA, cpsB)

                pending_norm = (oo, cpsA, cpsB)
                if oo + 1 < NP:
                    cur = nxt
            stage7 = emit_norm(*pending_norm)

            # warm the Square/Sqrt tables right after the last exp (the
            # data dep on expS pins these behind the attention in ACT order)
            sqwarm = consts.tile([P, 1], f32)
            nc.scalar.activation(
                out=sqwarm, in_=expS[:, KT8 - 1, 1, 0:1], func=Act.Square
            )
            nc.scalar.activation(out=sqwarm, in_=sqwarm, func=Act.Sqrt)

            # ---- second-half output projection + layernorm tail ----
            # halfB matmuls land in sc_ps [P,2,QR] tiles; DR(4,5) groups are
            # emitted before DR(6,7) so only the latter waits on the pair-7
            # stage move.  Drains: DVE w/ accum (st0/1), ACT-copy+Pool-add
            # (st2/3, Pool cannot touch PSUM; musum from precomputed ya
            # sums).  sumsq via ACT Square+accum.  Output bf16.
            # pass 1: all four output-projection psums + DVE drains, so
            # the drains sit at the FRONT of DVE's in-order tail queue
            musums = []
            for st in range(ST):
                sps = sc_ps.tile([P, 2, QR], f32, tag="sc", name="obps")
                for oh in range(2):
                    nc.tensor.matmul(
                        sps[:, oh, :],
                        ctxT[:, 4:6, st * P : (st + 1) * P],
                        owt[:, 4:6, oh * QR : (oh + 1) * QR],
                        start=True, stop=False, perf_mode=DR,
                    )
                for oh in range(2):
                    nc.tensor.matmul(
                        sps[:, oh, :],
                        ctxT[:, 6, st * P : (st + 1) * P],
                        owt[:, 6, oh * QR : (oh + 1) * QR],
                        start=False, stop=False,
                    )
                for oh in range(2):
                    nc.tensor.matmul(
                        sps[:, oh, :],
                        ctxT[0:HS, 7, st * P : (st + 1) * P],
                        owt[0:HS, 7, oh * QR : (oh + 1) * QR],
                        start=False, stop=False,
                    )
                for oh in range(2):
                    nc.tensor.matmul(
                        sps[:, oh, :],
                        stage7[:, st * P : (st + 1) * P],
                        owt7o[:, oh * QR : (oh + 1) * QR],
                        start=False, stop=True,
                    )
                musum = small.tile([P, 1], f32, tag="musum")
                nc.vector.scalar_tensor_tensor(
                    out=ybf[:, st, :].rearrange("p (j q) -> p j q", q=QR),
                    in0=sps,
                    scalar=0.0,
                    in1=ya[:, st, :].rearrange("p (j q) -> p j q", q=QR),
                    op0=Alu.add,
                    op1=Alu.add,
                    accum_out=musum,
                )
                musums.append(musum)

            # pass 2: per-st stats + normalize + affine + store
            for st in range(ST):
                musum = musums[st]
                scr = onorm_pool.tile([P, H], bf16, tag="scr")
                ysq = small.tile([P, 1], f32, tag="ysq")
                nc.scalar.activation(
                    out=scr, in_=ybf[:, st, :], func=Act.Square,
                    accum_out=ysq,
                )
                mu = small.tile([P, 1], f32, tag="mu")
                nc.vector.tensor_scalar(
                    out=mu, in0=musum, scalar1=rH, scalar2=None, op0=Alu.mult,
                )
                mu2 = small.tile([P, 1], f32, tag="mu2")
                nc.vector.tensor_tensor(out=mu2, in0=mu, in1=mu, op=Alu.mult)
                var = small.tile([P, 1], f32, tag="var")
                nc.vector.tensor_scalar(
                    out=var, in0=ysq, scalar1=rH, scalar2=mu2,
                    op0=Alu.mult, op1=Alu.subtract,
                )
                rstd = small.tile([P, 1], f32, tag="rstd")
                nc.scalar.activation(
                    out=rstd, in_=var, func=Act.Sqrt, bias=eps_sb, scale=1.0
                )
                nc.vector.reciprocal(out=rstd, in_=rstd)
                on = onorm_pool.tile([P, H], bf16, tag="on")
                nc.vector.tensor_scalar(
                    out=on, in0=ybf[:, st, :], scalar1=mu, scalar2=rstd,
                    op0=Alu.subtract, op1=Alu.mult,
                )
                nc.vector.tensor_tensor(out=on, in0=on, in1=gbb_sb, op=Alu.mult)
                onf = onorm_pool.tile([P, H], bf16, tag="onf")
                nc.vector.tensor_tensor(out=onf, in0=on, in1=bbb_sb, op=Alu.add)
                oeng = nc.scalar if st % 2 else nc.sync
                oeng.dma_start(
                    out=out_d.rearrange("(st p) m -> p st m", p=P)[:, st, :],
                    in_=onf,
                )

    nc.compile()
    return nc


def _get_nc():
    if "nc" not in _CACHE:
        _CACHE["nc"] = _build_nc()
    return _CACHE["nc"]


def _make_in_maps(inputs):
    import ml_dtypes

    f8 = ml_dtypes.float8_e4m3
    bf = ml_dtypes.bfloat16
    hs = np.asarray(inputs["hidden_states"], dtype=np.float32).reshape(B, S, H)
    am = np.asarray(inputs["attention_mask"], dtype=np.float32).reshape(B, S)

    # shared fp8 weight block [4096, 1024]: qw^T, kw^T, vw^T, ow^T
    wblk = np.empty((4 * H, H), dtype=f8)
    for i, nm in enumerate(("qw", "kw", "vw", "ow")):
        wblk[i * H : (i + 1) * H] = np.asarray(inputs[nm], np.float32).T.astype(f8)

    auxf_shared = np.zeros((AUXF_ROWS, H), dtype=np.float32)
    auxf_shared[R_VB] = np.asarray(inputs["vb"], np.float32)
    auxf_shared[R_OB] = np.asarray(inputs["ob"], np.float32)
    qb = np.asarray(inputs["qb"], np.float32)
    kb = np.asarray(inputs["kb"], np.float32)
    gb = np.asarray(inputs["gamma"], np.float32).astype(bf)
    bb = np.asarray(inputs["beta"], np.float32).astype(bf)

    in_maps = []
    for c in range(N_CORES):
        b, half = divmod(c, 2)
        x = hs[b]
        m = am[b]
        if half:
            x = np.roll(x, -QR, axis=0)
            m = np.roll(m, -QR)
        wx = np.empty((5 * H, S), dtype=f8)
        wx[0:H] = x.T.astype(f8)
        wx[H:] = wblk
        # pretransposed [128, 3*8] qkm block: [p, r, io] = row_r[io*128+p]
        qkm = np.empty((P, 3, H // P), dtype=np.float32)
        qkm[:, 0, :] = qb.reshape(H // P, P).T
        qkm[:, 1, :] = kb.reshape(H // P, P).T
        qkm[:, 2, :] = m.reshape(H // P, P).T
        xb = np.empty((XB_ROWS, H), dtype=bf)
        xb[0:QR] = x[:QR].astype(bf)
        xb[R_GAMMA] = gb
        xb[R_BETA] = bb
        in_maps.append({
            "wx": wx,
            "qkm": qkm.reshape(P, 3 * (H // P)),
            "auxf": auxf_shared,
            "xb": xb,
        })
    return in_maps


def _gather(results):
    out = np.empty((B, S, H), dtype=np.float32)
    for c in range(N_CORES):
        b, half = divmod(c, 2)
        out[b, half * QR : (half + 1) * QR, :] = results[c]["out"]
    return out


def run_on_hw(inputs, **kwargs):
    """Run on the 8 NeuronCores; returns (full_output, BassKernelResults)."""
    from concourse import bass_utils

    nc = _get_nc()
    in_maps = _make_in_maps(inputs)
    res = bass_utils.run_bass_kernel_spmd(
        nc, in_maps, core_ids=list(range(N_CORES)), **kwargs
    )
    return _gather(res.results), res


def kernel(**inputs) -> np.ndarray:
    out, _ = run_on_hw(inputs)
    return out


# revision 6
# speedup vs baseline: 5.3348x; 5.0394x over previous
"""BERT attention layer (B=4, S=1024, H=1024, NH=16) on 8 TRN2 NeuronCores.

Sharding: core c -> batch b = c//2, sequence-half = c%2 (host rolls the
sequence for odd cores so one SPMD program serves all 8 cores).

Design (vs the 139us-sim / 184us-HW baseline):
- ACT is the pacing engine (64 exp tiles x ~1us = 66us).  Everything else
  is arranged to keep ACT gapless: chunked priority-ordered input DMAs
  (first exp ~7us), four score steps always in flight, the previous
  pair's normalize deferred until after the next pair's first scores.
- Both heads' ctx run aug'd DoubleRow matmuls at psum base 0 (DR dst
  must be base 0; the V ones-column lands each softmax denominator at
  psum row 64 for free).  Normalize = 2 bf16 psum->sbuf copies, 1 bf16
  reciprocal, 2 tiny bf16 broadcast matmuls, 2 multiplies; the odd
  half moves to ctxT rows 64:128 via an SBUF->SBUF DMA on the idle SP
  queue (engines cannot cross partitions; Pool cannot touch PSUM).
- Output projection split: j=0:512 contracts during pairs 4-7 into SBUF
  ya with residual+output-bias folded in and row-sums accumulated; only
  j=512:1024 remains after attention, in [P,2,QR] score-pool psums.
- bf16 LN tail split across engines: psum drains on DVE (st0/1, with
  accum_out row-sums) and ACT-copy+Pool-add (st2/3); sumsq via ACT
  Square+accum_out; (y-mu)*rstd, gamma, beta on DVE in bf16 (2x/4x
  modes); output tensor is bf16 (host converts to f32 in the gather).
"""

import sys

for _p in ("/opt/trn_rl_repo", "/root/.axon_site/_ro/trn_rl_repo"):
    if _p not in sys.path:
        sys.path.insert(0, _p)

import numpy as np

B, S, H, NH, HS = 4, 1024, 1024, 16, 64
P = 128
QR = 512          # query rows per core
EPS = 1e-12
N_CORES = 8
NP = NH // 2      # head pairs

# f32 aux rows
R_VB, R_OB = 0, 1
AUXF_ROWS = 2
# bf16 bundle rows: residual rows then gamma, beta
R_GAMMA = QR
R_BETA = QR + 1
XB_ROWS = QR + 2

_CACHE = {}


def _build_nc():
    import concourse.mybir as mybir
    import concourse.tile as tile
    from concourse import bacc

    f32 = mybir.dt.float32
    bf16 = mybir.dt.bfloat16
    f8 = mybir.dt.float8e4
    DR = mybir.MatmulPerfMode.DoubleRow
    Alu = mybir.AluOpType
    Act = mybir.ActivationFunctionType

    nc = bacc.Bacc("TRN2", target_bir_lowering=False)

    # ---- packed per-core DRAM I/O ----
    wx_d = nc.declare_dram_parameter("wx", [5 * H, S], f8, isOutput=False)
    # host-pretransposed [128, 3*8] qb/kb/mask block (contiguous lines)
    qkm_d = nc.declare_dram_parameter("qkm", [P, 3 * (H // P)], f32, isOutput=False)
    auxf_d = nc.declare_dram_parameter("auxf", [AUXF_ROWS, H], f32, isOutput=False)
    xb_d = nc.declare_dram_parameter("xb", [XB_ROWS, H], bf16, isOutput=False)
    out_d = nc.declare_dram_parameter("out", [QR, H], bf16, isOutput=True)

    xt_d = wx_d[0:H, :]
    qwt_d = wx_d[H : 2 * H, :]
    kwt_d = wx_d[2 * H : 3 * H, :]
    vwt_d = wx_d[3 * H : 4 * H, :]
    owt_d = wx_d[4 * H : 5 * H, :]

    KT8 = H // P   # 8 tiles along any 1024 dim
    ST = QR // P   # 4 row-tiles of the output

    with tile.TileContext(nc) as tc:
        with (
            tc.tile_pool(name="consts", bufs=1) as consts,
            tc.tile_pool(name="bigs", bufs=1) as bigs,
            tc.tile_pool(name="pers", bufs=2) as pers,
            tc.tile_pool(name="wbuf", bufs=4) as wbuf,
            tc.tile_pool(name="exps", bufs=2) as exps,
            tc.tile_pool(name="small", bufs=8) as small,
            tc.tile_pool(name="onorm", bufs=3) as onorm_pool,
            tc.tile_pool(name="norm", bufs=2) as norm_pool,
            tc.tile_pool(name="mm_ps", bufs=2, space="PSUM") as mm_ps,
            tc.tile_pool(name="sc_ps", bufs=2, space="PSUM") as sc_ps,
            tc.tile_pool(name="ctx_ps", bufs=1, space="PSUM") as ctx_ps,
            tc.tile_pool(name="dram", bufs=2, space="DRAM") as dram_pool,
        ):
            # ---- input DMAs.  HWDGE and the DMA engines are shared serial
            # resources; SP carries the critical sequence in strict
            # first-use order, ACT only the tiny qkm block.  128-col head
            # chunks of qwt/kwt unblock pair 0 (it only reads out-tile 0).
            qkm_sb = consts.tile([P, 3, KT8], f32)
            nc.scalar.dma_start(
                out=qkm_sb, in_=qkm_d.rearrange("p (r io) -> p r io", r=3)
            )
            qb_sb = qkm_sb[:, 0, :]
            kb_sb = qkm_sb[:, 1, :]
            mask_sb = qkm_sb[:, 2, :]

            # q/k projections and the sh=0 K half only read xt cols 0:512,
            # so that half loads first
            xt_sb = pers.tile([P, KT8, S], f8, tag="xt")
            xt_r = xt_d.rearrange("(io p) s -> p io s", p=P)
            qwt = wbuf.tile([P, KT8, H], f8, tag="wt", name="qwt")
            qwt_r = qwt_d.rearrange("(io p) o -> p io o", p=P)
            kwt = wbuf.tile([P, KT8, H], f8, tag="wt", name="kwt")
            kwt_r = kwt_d.rearrange("(io p) o -> p io o", p=P)
            nc.sync.dma_start(out=xt_sb[:, :, 0:QR], in_=xt_r[:, :, 0:QR])
            nc.sync.dma_start(out=qwt[:, :, 0:P], in_=qwt_r[:, :, 0:P])
            nc.sync.dma_start(out=kwt[:, :, 0:P], in_=kwt_r[:, :, 0:P])
            nc.sync.dma_start(out=kwt[:, :, P : 2 * P], in_=kwt_r[:, :, P : 2 * P])
            nc.sync.dma_start(out=qwt[:, :, P : 2 * P], in_=qwt_r[:, :, P : 2 * P])
            nc.sync.dma_start(out=xt_sb[:, :, QR:S], in_=xt_r[:, :, QR:S])
            nc.sync.dma_start(out=kwt[:, :, 2 * P : H], in_=kwt_r[:, :, 2 * P : H])
            nc.sync.dma_start(out=qwt[:, :, 2 * P : H], in_=qwt_r[:, :, 2 * P : H])
            vwt = wbuf.tile([P, KT8, H], f8, tag="wt", name="vwt")
            vwt_r = vwt_d.rearrange("(io p) o -> p io o", p=P)
            nc.sync.dma_start(out=vwt[:, :, 0:QR], in_=vwt_r[:, :, 0:QR])
            nc.sync.dma_start(out=vwt[:, :, QR:S], in_=vwt_r[:, :, QR:S])
            vbb_sb = consts.tile([P, H], f32)
            nc.sync.dma_start(
                out=vbb_sb, in_=auxf_d[R_VB : R_VB + 1, :].to_broadcast([P, H])
            )
            owt = wbuf.tile([P, KT8, H], f8, tag="wt", name="owt")
            owt_r = owt_d.rearrange("(io p) o -> p io o", p=P)
            for c in range(2):
                io = slice(4 * c, 4 * c + 4)
                nc.sync.dma_start(out=owt[:, io, :], in_=owt_r[:, io, :])
            # io-7 odd-head weight rows relocated to base partition 0 so the
            # tail can contract straight from the pair-7 staging tile
            owt7o = consts.tile([HS, H], f8)
            nc.sync.dma_start(out=owt7o, in_=owt_r[HS:P, 7, :])
            xres_r = xb_d[0:QR, :].rearrange("(st p) m -> p st m", p=P)
            xres_sb = bigs.tile([P, ST, H], bf16)
            for st in range(ST):
                nc.sync.dma_start(out=xres_sb[:, st, :], in_=xres_r[:, st, :])
            obb_sb = consts.tile([P, H], f32)
            nc.sync.dma_start(
                out=obb_sb, in_=auxf_d[R_OB : R_OB + 1, :].to_broadcast([P, H])
            )
            gbb_sb = consts.tile([P, H], bf16)
            nc.sync.dma_start(
                out=gbb_sb, in_=xb_d[R_GAMMA : R_GAMMA + 1, :].to_broadcast([P, H])
            )
            bbb_sb = consts.tile([P, H], bf16)
            nc.sync.dma_start(
                out=bbb_sb, in_=xb_d[R_BETA : R_BETA + 1, :].to_broadcast([P, H])
            )

            # ---- small constants ----
            eps_sb = consts.tile([P, 1], f32)
            nc.vector.memset(eps_sb, EPS)
            ones_bc = consts.tile([P, HS], bf16)
            nc.vector.memset(ones_bc, 1.0)
            rH = consts.tile([P, 1], f32)
            nc.vector.memset(rH, 1.0 / H)
            # warm the ACT exp table so the first real exp doesn't pay the
            # table load
            actwarm = consts.tile([P, 1], f32)
            nc.scalar.activation(out=actwarm, in_=eps_sb, func=Act.Exp)

            # ---- persistent big tensors ----
            QT = pers.tile([P, KT8, QR], bf16, tag="QT")   # Q^T  [o, q]
            KT = bigs.tile([P, KT8, S], bf16)              # K^T  [o, s]
            Vaug = bigs.tile([P, KT8, NH, HS + 1], f8)     # V [s, h, d(+ones)]
            ctxT = bigs.tile([P, KT8, QR], f8)             # ctx^T [j, q]
            ya = bigs.tile([P, ST, H], f32)                # first-half out-proj
            yasum = bigs.tile([P, ST, 2], f32)             # row-sums of ya halves
            xo = bigs.tile([P, ST, H], f32)                # residual + ob
            ybf = bigs.tile([P, ST, H], bf16)              # final y (bf16)

            nc.vector.memset(Vaug[:, :, :, HS : HS + 1], 1.0)

            def emit_v(st, oh):
                ps = mm_ps.tile([P, QR], f32, tag="mm", name="vps")
                for it in range(KT8 // 2):
                    nc.tensor.matmul(
                        ps,
                        xt_sb[:, 2 * it : 2 * it + 2, st * P : (st + 1) * P],
                        vwt[:, 2 * it : 2 * it + 2, oh * QR : (oh + 1) * QR],
                        start=(it == 0),
                        stop=(it == KT8 // 2 - 1),
                        perf_mode=DR,
                    )
                nc.vector.tensor_tensor(
                    out=Vaug[:, st, oh * (NH // 2) : (oh + 1) * (NH // 2), 0:HS],
                    in0=ps.rearrange("p (h d) -> p h d", d=HS),
                    in1=vbb_sb[:, oh * QR : (oh + 1) * QR].rearrange(
                        "p (h d) -> p h d", d=HS
                    ),
                    op=Alu.add,
                )

            def emit_q(ot):
                ps = mm_ps.tile([P, QR], f32, tag="mm", name="qps")
                for it in range(KT8 // 2):
                    nc.tensor.matmul(
                        ps,
                        qwt[:, 2 * it : 2 * it + 2, ot * P : (ot + 1) * P],
                        xt_sb[:, 2 * it : 2 * it + 2, 0:QR],
                        start=(it == 0),
                        stop=(it == KT8 // 2 - 1),
                        perf_mode=DR,
                    )
                nc.vector.tensor_scalar_add(
                    out=QT[:, ot, :], in0=ps, scalar1=qb_sb[:, ot : ot + 1]
                )

            def emit_kt(ot, sh):
                kps = mm_ps.tile([P, QR], f32, tag="mm", name="kps")
                for it in range(KT8 // 2):
                    nc.tensor.matmul(
                        kps,
                        kwt[:, 2 * it : 2 * it + 2, ot * P : (ot + 1) * P],
                        xt_sb[:, 2 * it : 2 * it + 2, sh * QR : (sh + 1) * QR],
                        start=(it == 0),
                        stop=(it == KT8 // 2 - 1),
                        perf_mode=DR,
                    )
                nc.vector.tensor_scalar_add(
                    out=KT[:, ot, sh * QR : (sh + 1) * QR],
                    in0=kps,
                    scalar1=kb_sb[:, ot : ot + 1],
                )

            def emit_xo(st):
                nc.gpsimd.tensor_tensor(
                    out=xo[:, st, :], in0=xres_sb[:, st, :], in1=obb_sb,
                    op=Alu.add,
                )

            def emit_oprojA(st, oh):
                ps = mm_ps.tile([P, QR], f32, tag="mm", name="oaps")
                for jo in range(2):
                    nc.tensor.matmul(
                        ps,
                        ctxT[:, 2 * jo : 2 * jo + 2, st * P : (st + 1) * P],
                        owt[:, 2 * jo : 2 * jo + 2, oh * QR : (oh + 1) * QR],
                        start=(jo == 0),
                        stop=(jo == 1),
                        perf_mode=DR,
                    )
                nc.vector.scalar_tensor_tensor(
                    out=ya[:, st, oh * QR : (oh + 1) * QR],
                    in0=ps,
                    scalar=0.0,
                    in1=xo[:, st, oh * QR : (oh + 1) * QR],
                    op0=Alu.add,
                    op1=Alu.add,
                    accum_out=yasum[:, st, oh : oh + 1],
                )

            def alloc_pair():
                expS = exps.tile([P, KT8, 2, QR], f8, tag="expS", name="expS")
                cpsA = ctx_ps.tile([P, QR], f32, tag="ctxA", name="cpsA")
                cpsB = ctx_ps.tile([P, QR], f32, tag="ctxB", name="cpsB")
                return expS, cpsA, cpsB

            def emit_scores(oo, so, expS):
                sps = sc_ps.tile([P, 2, QR], f32, tag="sc", name="sps")
                for j in range(2):
                    po = j * HS
                    nc.tensor.matmul(
                        sps[:, j, :],
                        KT[po : po + HS, oo, so * P : (so + 1) * P],
                        QT[po : po + HS, oo, :],
                        start=True,
                        stop=True,
                    )
                nc.scalar.activation(
                    out=expS[:, so, :, :],
                    in_=sps,
                    func=Act.Exp,
                    bias=mask_sb[:, so : so + 1],
                    scale=0.125,
                )

            def emit_ctx_pair(oo, t, expS, cpsA, cpsB):
                first, last = t == 0, t == KT8 // 2 - 1
                # both heads: aug'd DoubleRow at base 0 (DR dst must be base
                # 0); the ones column lands each head's softmax denominator
                # at psum row 64 for free
                for j in range(2):
                    nc.tensor.matmul(
                        (cpsA, cpsB)[j][0 : HS + 1, :],
                        Vaug[:, 2 * t : 2 * t + 2, 2 * oo + j, :],
                        expS[:, 2 * t : 2 * t + 2, j, :],
                        start=first, stop=last, perf_mode=DR,
                    )

            def emit_norm(oo, cpsA, cpsB):
                # copy both heads' ctx+denominator rows to SBUF (bf16),
                # reciprocal, broadcast via tiny bf16 matmuls into mm-pool
                # psums, multiply; the odd half moves to ctxT rows 64:128
                # via an SBUF->SBUF DMA on the otherwise-idle SP queue
                cc = norm_pool.tile([HS + 1, 2, QR], bf16, tag="cc")
                nc.vector.tensor_copy(out=cc[:, 0, :], in_=cpsA[0 : HS + 1, :])
                nc.vector.tensor_copy(out=cc[:, 1, :], in_=cpsB[0 : HS + 1, :])
                rdt = norm_pool.tile([P, 2, QR], bf16, tag="rd2")
                with nc.allow_low_precision(
                    "softmax denominators; bf16 recip adds <0.5% rel err"
                ):
                    nc.vector.reciprocal(
                        out=rdt[HS : HS + 1, :, :], in_=cc[HS : HS + 1, :, :]
                    )
                bcA = mm_ps.tile([P, QR], f32, tag="mm", name="bcA")
                bcB = mm_ps.tile([P, QR], f32, tag="mm", name="bcB")
                nc.tensor.matmul(
                    bcA[0:HS, :], ones_bc[HS : HS + 1, :], rdt[HS : HS + 1, 0, :],
                    start=True, stop=True,
                )
                nc.tensor.matmul(
                    bcB[0:HS, :], ones_bc[HS : HS + 1, :], rdt[HS : HS + 1, 1, :],
                    start=True, stop=True,
                )
                nc.vector.tensor_tensor(
                    out=ctxT[0:HS, oo, :], in0=cc[0:HS, 0, :], in1=bcA[0:HS, :],
                    op=Alu.mult,
                )
                stage = norm_pool.tile([HS, QR], f8, tag="stage", bufs=3)
                nc.vector.tensor_tensor(
                    out=stage, in0=cc[0:HS, 1, :], in1=bcB[0:HS, :],
                    op=Alu.mult,
                )
                if oo < NP - 1:
                    nc.sync.dma_start(out=ctxT[HS:P, oo, :], in_=stage)
                return stage

            # ---- attention ----
            emit_q(0)
            emit_kt(0, 0)

            # PE queue is in-order: q(0)/kt(0,0), then the first four score
            # steps (they only need out-tile 0), then the rest of the
            # preamble that waits on later DMA chunks
            cur = alloc_pair()
            for so in range(4):
                emit_scores(0, so, cur[0])
            emit_kt(0, 1)
            emit_q(1)
            pending_norm = None

            extra = {}

            def add_extra(oo, fn, args, front=False):
                lst = extra.setdefault(oo, [])
                lst.insert(0, (fn, args)) if front else lst.append((fn, args))

            for i, st in enumerate(range(KT8)):
                add_extra(1 + i // 3, emit_v, (st, 1))
            # K/Q for the NEXT pairs must be emitted BEFORE that pair's
            # pre-emitted score matmuls (program order defines data flow),
            # so they go at the FRONT of the work list (chunk 0)
            for oo in range(1, NP - 2):
                add_extra(oo, emit_q, (oo + 2,), front=True)
            for oo in range(1, NP - 1):
                add_extra(oo, emit_kt, (oo + 1, 1), front=True)
                add_extra(oo, emit_kt, (oo + 1, 0), front=True)
            for st in range(ST):
                for oh in range(2):
                    add_extra(4 + st, emit_oprojA, (st, oh))
            # pair 0 interleave is hand-ordered: ctx(0,t) needs emit_v(2t,0)
            # and emit_v(2t+1,0) by chunk t, and kt(1,0) must precede the
            # pre-emitted pair-1 scores
            extra[0] = [
                (emit_v, (0, 0)), (emit_v, (1, 0)), (emit_kt, (1, 0)),
                (emit_v, (2, 0)), (emit_v, (3, 0)), (emit_kt, (1, 1)),
                (emit_v, (4, 0)), (emit_v, (5, 0)), (emit_q, (2,)),
                (emit_v, (6, 0)), (emit_v, (7, 0)),
            ]

            for oo in range(NP):
                expS, cpsA, cpsB = cur
                work = list(extra.get(oo, []))
                if oo in (2, 3):
                    work.append((emit_xo, (2 * (oo - 2),)))
                    work.append((emit_xo, (2 * (oo - 2) + 1,)))
                chunks = [[] for _ in range(4)]
                for i, w in enumerate(work):
                    chunks[i * 4 // max(len(work), 1)].append(w)

                for t in range(KT8 // 2):
                    if t < 2:
                        emit_scores(oo, 2 * t + 4, expS)
                        emit_scores(oo, 2 * t + 5, expS)
                    elif oo + 1 < NP:
                        if t == 2:
                            nxt = alloc_pair()
                        emit_scores(oo + 1, 2 * (t - 2), nxt[0])
                        emit_scores(oo + 1, 2 * (t - 2) + 1, nxt[0])
                    for fn, args in chunks[t]:
                        fn(*args)
                    if t == 0 and pending_norm is not None:
                        # previous pair's normalize after this pair's first
                        # scores AND chunk-0 work (kt/q emits), but before
                        # ctx(0) which reuses the psum banks
                        emit_norm(*pending_norm)
                    emit_ctx_pair(oo, t, expS, cpsA, cpsB)

                pending_norm = (oo, cpsA, cpsB)
                if oo + 1 < NP:
                    cur = nxt
            stage7 = emit_norm(*pending_norm)

            # warm the Square/Sqrt tables right after the last exp (the
            # data dep on expS pins these behind the attention in ACT order)
            sqwarm = consts.tile([P, 1], f32)
            nc.scalar.activation(
                out=sqwarm, in_=expS[:, KT8 - 1, 1, 0:1], func=Act.Square
            )
            nc.scalar.activation(out=sqwarm, in_=sqwarm, func=Act.Sqrt)

            # ---- second-half output projection + layernorm tail ----
            # halfB matmuls land in sc_ps [P,2,QR] tiles; DR(4,5) groups are
            # emitted before DR(6,7) so only the latter waits on the pair-7
            # stage move.  Drains: DVE w/ accum (st0/1), ACT-copy+Pool-add
            # (st2/3, Pool cannot touch PSUM; musum from precomputed ya
            # sums).  sumsq via ACT Square+accum.  Output bf16.
            # pass 1: all four output-projection psums + DVE drains, so
            # the drains sit at the FRONT of DVE's in-order tail queue
            musums = []
            for st in range(ST):
                sps = sc_ps.tile([P, 2, QR], f32, tag="sc", name="obps")
                for oh in range(2):
                    nc.tensor.matmul(
                        sps[:, oh, :],
                        ctxT[:, 4:6, st * P : (st + 1) * P],
                        owt[:, 4:6, oh * QR : (oh + 1) * QR],
                        start=True, stop=False, perf_mode=DR,
                    )
                for oh in range(2):
                    nc.tensor.matmul(
                        sps[:, oh, :],
                        ctxT[:, 6, st * P : (st + 1) * P],
                        owt[:, 6, oh * QR : (oh + 1) * QR],
                        start=False, stop=False,
                    )
                for oh in range(2):
                    nc.tensor.matmul(
                        sps[:, oh, :],
                        ctxT[0:HS, 7, st * P : (st + 1) * P],
                        owt[0:HS, 7, oh * QR : (oh + 1) * QR],
                        start=False, stop=False,
                    )
                for oh in range(2):
                    nc.tensor.matmul(
                        sps[:, oh, :],
                        stage7[:, st * P : (st + 1) * P],
                        owt7o[:, oh * QR : (oh + 1) * QR],
                        start=False, stop=True,
                    )
                musum = small.tile([P, 1], f32, tag="musum")
                nc.vector.scalar_tensor_tensor(
                    out=ybf[:, st, :].rearrange("p (j q) -> p j q", q=QR),
                    in0=sps,
                    scalar=0.0,
                    in1=ya[:, st, :].rearrange("p (j q) -> p j q", q=QR),
                    op0=Alu.add,
                    op1=Alu.add,
                    accum_out=musum,
                )
                musums.append(musum)

            # pass 2: per-st stats + normalize + affine + store
            for st in range(ST):
                musum = musums[st]
                scr = onorm_pool.tile([P, H], bf16, tag="scr")
                ysq = small.tile([P, 1], f32, tag="ysq")
                nc.scalar.activation(
                    out=scr, in_=ybf[:, st, :], func=Act.Square,
                    accum_out=ysq,
                )
                mu = small.tile([P, 1], f32, tag="mu")
                nc.vector.tensor_scalar(
                    out=mu, in0=musum, scalar1=rH, scalar2=None, op0=Alu.mult,
                )
                mu2 = small.tile([P, 1], f32, tag="mu2")
                nc.vector.tensor_tensor(out=mu2, in0=mu, in1=mu, op=Alu.mult)
                var = small.tile([P, 1], f32, tag="var")
                nc.vector.tensor_scalar(
                    out=var, in0=ysq, scalar1=rH, scalar2=mu2,
                    op0=Alu.mult, op1=Alu.subtract,
                )
                rstd = small.tile([P, 1], f32, tag="rstd")
                nc.scalar.activation(
                    out=rstd, in_=var, func=Act.Sqrt, bias=eps_sb, scale=1.0
                )
                nc.vector.reciprocal(out=rstd, in_=rstd)
                on = onorm_pool.tile([P, H], bf16, tag="on")
                nc.vector.tensor_scalar(
                    out=on, in0=ybf[:, st, :], scalar1=mu, scalar2=rstd,
                    op0=Alu.subtract, op1=Alu.mult,
                )
                nc.vector.tensor_tensor(out=on, in0=on, in1=gbb_sb, op=Alu.mult)
                onf = onorm_pool.tile([P, H], bf16, tag="onf")
                nc.vector.tensor_tensor(out=onf, in0=on, in1=bbb_sb, op=Alu.add)
                oeng = nc.scalar if st % 2 else nc.sync
                oeng.dma_start(
                    out=out_d.rearrange("(st p) m -> p st m", p=P)[:, st, :],
                    in_=onf,
                )

    nc.compile()
    return nc


def _get_nc():
    if "nc" not in _CACHE:
        _CACHE["nc"] = _build_nc()
    return _CACHE["nc"]


def _make_in_maps(inputs):
    import ml_dtypes

    f8 = ml_dtypes.float8_e4m3
    bf = ml_dtypes.bfloat16
    hs = np.asarray(inputs["hidden_states"], dtype=np.float32).reshape(B, S, H)
    am = np.asarray(inputs["attention_mask"], dtype=np.float32).reshape(B, S)

    # shared fp8 weight block [4096, 1024]: qw^T, kw^T, vw^T, ow^T
    wblk = np.empty((4 * H, H), dtype=f8)
    for i, nm in enumerate(("qw", "kw", "vw", "ow")):
        wblk[i * H : (i + 1) * H] = np.asarray(inputs[nm], np.float32).T.astype(f8)

    auxf_shared = np.zeros((AUXF_ROWS, H), dtype=np.float32)
    auxf_shared[R_VB] = np.asarray(inputs["vb"], np.float32)
    auxf_shared[R_OB] = np.asarray(inputs["ob"], np.float32)
    qb = np.asarray(inputs["qb"], np.float32)
    kb = np.asarray(inputs["kb"], np.float32)
    gb = np.asarray(inputs["gamma"], np.float32).astype(bf)
    bb = np.asarray(inputs["beta"], np.float32).astype(bf)

    in_maps = []
    for c in range(N_CORES):
        b, half = divmod(c, 2)
        x = hs[b]
        m = am[b]
        if half:
            x = np.roll(x, -QR, axis=0)
            m = np.roll(m, -QR)
        wx = np.empty((5 * H, S), dtype=f8)
        wx[0:H] = x.T.astype(f8)
        wx[H:] = wblk
        # pretransposed [128, 3*8] qkm block: [p, r, io] = row_r[io*128+p]
        qkm = np.empty((P, 3, H // P), dtype=np.float32)
        qkm[:, 0, :] = qb.reshape(H // P, P).T
        qkm[:, 1, :] = kb.reshape(H // P, P).T
        qkm[:, 2, :] = m.reshape(H // P, P).T
        xb = np.empty((XB_ROWS, H), dtype=bf)
        xb[0:QR] = x[:QR].astype(bf)
        xb[R_GAMMA] = gb
        xb[R_BETA] = bb
        in_maps.append({
            "wx": wx,
            "qkm": qkm.reshape(P, 3 * (H // P)),
            "auxf": auxf_shared,
            "xb": xb,
        })
    return in_maps


def _gather(results):
    out = np.empty((B, S, H), dtype=np.float32)
    for c in range(N_CORES):
        b, half = divmod(c, 2)
        out[b, half * QR : (half + 1) * QR, :] = results[c]["out"]
    return out


def run_on_hw(inputs, **kwargs):
    """Run on the 8 NeuronCores; returns (full_output, BassKernelResults)."""
    from concourse import bass_utils

    nc = _get_nc()
    in_maps = _make_in_maps(inputs)
    res = bass_utils.run_bass_kernel_spmd(
        nc, in_maps, core_ids=list(range(N_CORES)), **kwargs
    )
    return _gather(res.results), res


def kernel(**inputs) -> np.ndarray:
    out, _ = run_on_hw(inputs)
    return out
